# revision 1
# baseline (speedup 1.0000x reference)
"""Bayesian-router MoE kernel for 8 Trainium2 NeuronCores.

Strategy (expert-parallel, per sharding hint):
  - Router moments / top-k / combine weights: tiny (B*F*E ~ 17 MFLOP), computed
    on host in float64 (min rank4/rank5 score gap is ~1.7e-4, far above fp32
    noise, so expert selection is stable vs the fp32 reference).
  - Token dispatch: host gathers each expert's routed tokens into a padded,
    transposed buffer XgT [F, CAP] (the host-side equivalent of the
    all-to-all; full I/O contract means shard/unshard happens on host).
  - Device: each of the 8 cores runs the 2-expert MLP on its gathered tokens,
    entirely in transposed form (A1T = relu(W1^T XgT + b1), YT = W2^T A1T + b2)
    so no on-device transposes are needed; weights stream as lhsT directly.
  - Combine: host scatter-adds w[t,e] * Y_e rows into the output (the
    cross-device reduction of the unshard step).
"""

import os
import numpy as np

NCORES = 8
P = 128
TOP_K = 4


# ---------------------------------------------------------------------------
# host-side routing (matches reference math; float64 for stable ordering)
# ---------------------------------------------------------------------------
def _routing(h, W_mu, b_mu, W_logvar, b_logvar):
    h64 = h.astype(np.float64)
    mu = h64 @ W_mu.T.astype(np.float64) + b_mu.astype(np.float64)
    var = (h64 * h64) @ np.exp(W_logvar.astype(np.float64)).T + np.exp(
        b_logvar.astype(np.float64)
    )
    var = np.maximum(var, 1e-12)
    tilde = mu / np.sqrt(1.0 + (np.pi / 8.0) * var)
    t = tilde - tilde.max(axis=1, keepdims=True)
    ex = np.exp(t)
    probs = ex / ex.sum(axis=1, keepdims=True)
    idx = np.argsort(-tilde, axis=1, kind="stable")[:, :TOP_K]
    w = np.take_along_axis(probs, idx, axis=1)
    w = w / np.maximum(w.sum(axis=1, keepdims=True), 1e-12)
    return idx, w


# ---------------------------------------------------------------------------
# device kernel: 2-expert MLP on pre-gathered transposed tokens
# ---------------------------------------------------------------------------
def _build_kernel(epc, F, H, C, cap, chunks):
    import concourse.mybir as mybir
    import concourse.tile as tile
    from concourse import bacc

    f32 = mybir.dt.float32
    FK, HK, CK = F // P, H // P, C // P

    nc = bacc.Bacc("TRN2", target_bir_lowering=False, debug=False,
                   num_devices=NCORES)

    xt = nc.dram_tensor("xt", [epc, F, cap], f32, kind="ExternalInput")
    w1 = nc.dram_tensor("w1", [epc, F, H], f32, kind="ExternalInput")
    w2 = nc.dram_tensor("w2", [epc, H, C], f32, kind="ExternalInput")
    b1 = nc.dram_tensor("b1", [P, epc, HK], f32, kind="ExternalInput")
    b2 = nc.dram_tensor("b2", [P, epc, CK], f32, kind="ExternalInput")
    yt = nc.dram_tensor("yt", [epc, C, cap], f32, kind="ExternalOutput")

    with tile.TileContext(nc) as tc:
        with (
            tc.tile_pool(name="consts", bufs=1) as consts,
            tc.tile_pool(name="w1pool", bufs=2) as w1pool,
            tc.tile_pool(name="w2pool", bufs=2) as w2pool,
            tc.tile_pool(name="xpool", bufs=2) as xpool,
            tc.tile_pool(name="apool", bufs=2) as apool,
            tc.tile_pool(name="ypool", bufs=2) as ypool,
            tc.tile_pool(name="psum", bufs=4, space="PSUM") as pp,
        ):
            b1s = consts.tile([P, epc, HK], f32)
            nc.gpsimd.dma_start(out=b1s[:], in_=b1[:])
            b2s = consts.tile([P, epc, CK], f32)
            nc.gpsimd.dma_start(out=b2s[:], in_=b2[:])

            for e in range(epc):
                xts = xpool.tile([P, FK, cap], f32, tag="xt")
                nc.sync.dma_start(
                    out=xts[:], in_=xt[e].rearrange("(k p) n -> p k n", p=P)
                )
                w1s = w1pool.tile([P, FK, H], f32, tag="w1")
                nc.sync.dma_start(
                    out=w1s[:], in_=w1[e].rearrange("(k p) m -> p k m", p=P)
                )
                w2s = w2pool.tile([P, HK, C], f32, tag="w2")
                nc.sync.dma_start(
                    out=w2s[:], in_=w2[e].rearrange("(k p) m -> p k m", p=P)
                )

                a1s = apool.tile([P, HK, cap], f32, tag="a1")
                yts = ypool.tile([P, CK, cap], f32, tag="yt")

                for n0, nsz in chunks:
                    # layer 1: A1T[m] = relu(sum_k W1[k,m]^T @ XgT[k] + b1[m])
                    for m in range(HK):
                        ps = pp.tile([P, 512], f32, tag="ps")
                        for k in range(FK):
                            nc.tensor.matmul(
                                ps[:, :nsz],
                                w1s[:, k, m * P:(m + 1) * P],
                                xts[:, k, n0:n0 + nsz],
                                start=(k == 0),
                                stop=(k == FK - 1),
                            )
                        nc.scalar.activation(
                            a1s[:, m, n0:n0 + nsz],
                            ps[:, :nsz],
                            mybir.ActivationFunctionType.Relu,
                            bias=b1s[:, e, m:m + 1],
                        )
                    # layer 2: YT[m] = sum_k W2[k,m]^T @ A1T[k] + b2[m]
                    for m in range(CK):
                        ps = pp.tile([P, 512], f32, tag="ps")
                        for k in range(HK):
                            nc.tensor.matmul(
                                ps[:, :nsz],
                                w2s[:, k, m * P:(m + 1) * P],
                                a1s[:, k, n0:n0 + nsz],
                                start=(k == 0),
                                stop=(k == HK - 1),
                            )
                        nc.vector.tensor_scalar_add(
                            yts[:, m, n0:n0 + nsz],
                            ps[:, :nsz],
                            b2s[:, e, m:m + 1],
                        )
                nc.sync.dma_start(
                    out=yt[e].rearrange("(k p) n -> p k n", p=P), in_=yts[:]
                )

    nc.compile()
    return nc


def _chunks(cap):
    n = (cap + 511) // 512
    base, rem = divmod(cap, n)
    out = []
    off = 0
    for i in range(n):
        sz = base + (1 if i < rem else 0)
        out.append((off, sz))
        off += sz
    return out


# ---------------------------------------------------------------------------
# entry point
# ---------------------------------------------------------------------------
def kernel(h, W_mu, b_mu, W_logvar, b_logvar, W1, b1, W2, b2):
    from concourse.bass_utils import run_bass_kernel_spmd

    h = np.ascontiguousarray(np.asarray(h, dtype=np.float32))
    W1 = np.asarray(W1, dtype=np.float32)
    b1 = np.asarray(b1, dtype=np.float32)
    W2 = np.asarray(W2, dtype=np.float32)
    b2 = np.asarray(b2, dtype=np.float32)

    B, F = h.shape
    E, _, H = W1.shape
    C = W2.shape[2]
    assert E % NCORES == 0
    epc = E // NCORES
    FK, HK, CK = F // P, H // P, C // P

    topk_idx, topk_w = _routing(
        np.asarray(h), np.asarray(W_mu), np.asarray(b_mu),
        np.asarray(W_logvar), np.asarray(b_logvar)
    )

    # per-expert token lists
    toks, poss = [], []
    for e in range(E):
        tok, pos = np.nonzero(topk_idx == e)
        toks.append(tok)
        poss.append(pos)
    maxcnt = max(len(t) for t in toks)
    cap = max(64, -(-maxcnt // 32) * 32)
    chunks = _chunks(cap)

    # gather/dispatch: XgT per expert, padded to cap
    xt = np.zeros((NCORES, epc, F, cap), np.float32)
    for e in range(E):
        cnt = len(toks[e])
        xt[e // epc, e % epc, :, :cnt] = h[toks[e]].T

    # biases pre-swizzled to [P, epc, K] so the device DMA is contiguous
    b1_sw = np.ascontiguousarray(
        b1.reshape(E, HK, P).transpose(2, 0, 1).reshape(P, NCORES, epc, HK)
        .transpose(1, 0, 2, 3)
    )  # [NCORES, P, epc, HK]
    b2_sw = np.ascontiguousarray(
        b2.reshape(E, CK, P).transpose(2, 0, 1).reshape(P, NCORES, epc, CK)
        .transpose(1, 0, 2, 3)
    )

    nc = _build_kernel(epc, F, H, C, cap, chunks)

    in_maps = []
    for c in range(NCORES):
        lo, hi = c * epc, (c + 1) * epc
        in_maps.append({
            "xt": xt[c],
            "w1": W1[lo:hi],
            "w2": W2[lo:hi],
            "b1": b1_sw[c],
            "b2": b2_sw[c],
        })

    trace = bool(os.environ.get("MOE_KERNEL_TRACE"))
    res = run_bass_kernel_spmd(nc, in_maps, list(range(NCORES)), trace=trace)
    global LAST_RESULTS
    LAST_RESULTS = res

    # combine: scatter-add weighted expert outputs
    out = np.zeros((B, C), np.float32)
    for e in range(E):
        cnt = len(toks[e])
        yte = res.results[e // epc]["yt"][e % epc]  # [C, cap]
        out[toks[e]] += (
            topk_w[toks[e], poss[e]].astype(np.float32)[:, None]
            * yte[:, :cnt].T
        )
    return out


LAST_RESULTS = None


# revision 3
# speedup vs baseline: 2.3968x; 2.3968x over previous
"""Bayesian-router MoE kernel for 8 Trainium2 NeuronCores.

Strategy (expert-parallel, per sharding hint):
  - Router moments / top-k / combine weights: tiny (B*F*E ~ 17 MFLOP), computed
    on host in float64 (min rank4/rank5 score gap is ~1.7e-4, far above fp32
    noise, so expert selection is stable vs the fp32 reference).
  - Token dispatch: host gathers each expert's routed tokens into a padded,
    transposed buffer XgT [F, CAP] (the host-side equivalent of the
    all-to-all; full I/O contract means shard/unshard happens on host).
  - Device: each of the 8 cores runs the 2-expert MLP on its gathered tokens,
    entirely in transposed form (A1T = relu(W1^T XgT + b1), YT = W2^T A1T + b2)
    so no on-device transposes are needed; weights stream as lhsT directly.
  - Combine: host scatter-adds w[t,e] * Y_e rows into the output (the
    cross-device reduction of the unshard step).
"""

import os
import numpy as np

NCORES = 8
P = 128
TOP_K = 4


# ---------------------------------------------------------------------------
# host-side routing (matches reference math; float64 for stable ordering)
# ---------------------------------------------------------------------------
def _routing(h, W_mu, b_mu, W_logvar, b_logvar):
    h64 = h.astype(np.float64)
    mu = h64 @ W_mu.T.astype(np.float64) + b_mu.astype(np.float64)
    var = (h64 * h64) @ np.exp(W_logvar.astype(np.float64)).T + np.exp(
        b_logvar.astype(np.float64)
    )
    var = np.maximum(var, 1e-12)
    tilde = mu / np.sqrt(1.0 + (np.pi / 8.0) * var)
    t = tilde - tilde.max(axis=1, keepdims=True)
    ex = np.exp(t)
    probs = ex / ex.sum(axis=1, keepdims=True)
    idx = np.argsort(-tilde, axis=1, kind="stable")[:, :TOP_K]
    w = np.take_along_axis(probs, idx, axis=1)
    w = w / np.maximum(w.sum(axis=1, keepdims=True), 1e-12)
    return idx, w


# ---------------------------------------------------------------------------
# device kernel: 2-expert MLP on pre-gathered transposed tokens
# ---------------------------------------------------------------------------
def _build_kernel(epc, F, H, C, cap, chunks):
    import concourse.mybir as mybir
    import concourse.tile as tile
    from concourse import bacc

    f32 = mybir.dt.float32
    f32r = mybir.dt.float32r
    FK, HK, CK = F // P, H // P, C // P

    nc = bacc.Bacc("TRN2", target_bir_lowering=False, debug=False,
                   num_devices=NCORES)

    xt = nc.dram_tensor("xt", [epc, F, cap], f32, kind="ExternalInput")
    w1 = nc.dram_tensor("w1", [epc, F, H], f32, kind="ExternalInput")
    w2 = nc.dram_tensor("w2", [epc, H, C], f32, kind="ExternalInput")
    b1 = nc.dram_tensor("b1", [P, epc, HK], f32, kind="ExternalInput")
    b2 = nc.dram_tensor("b2", [P, epc, CK], f32, kind="ExternalInput")
    yt = nc.dram_tensor("yt", [epc, C, cap], f32, kind="ExternalOutput")

    with tile.TileContext(nc) as tc:
        with (
            tc.tile_pool(name="consts", bufs=1) as consts,
            tc.tile_pool(name="w1pool", bufs=2) as w1pool,
            tc.tile_pool(name="w2pool", bufs=2) as w2pool,
            tc.tile_pool(name="xpool", bufs=2) as xpool,
            tc.tile_pool(name="apool", bufs=2) as apool,
            tc.tile_pool(name="ypool", bufs=2) as ypool,
            tc.tile_pool(name="psum", bufs=4, space="PSUM") as pp,
        ):
            b1s = consts.tile([P, epc, HK], f32)
            nc.gpsimd.dma_start(out=b1s[:], in_=b1[:])
            b2s = consts.tile([P, epc, CK], f32)
            nc.gpsimd.dma_start(out=b2s[:], in_=b2[:])

            for e in range(epc):
                xts = xpool.tile([P, FK, cap], f32r, tag="xt")
                nc.gpsimd.dma_start(
                    out=xts[:], in_=xt[e].rearrange("(k p) n -> p k n", p=P)
                )
                w1s = w1pool.tile([P, FK, H], f32r, tag="w1")
                nc.gpsimd.dma_start(
                    out=w1s[:], in_=w1[e].rearrange("(k p) m -> p k m", p=P)
                )
                w2s = w2pool.tile([P, HK, C], f32r, tag="w2")
                nc.gpsimd.dma_start(
                    out=w2s[:], in_=w2[e].rearrange("(k p) m -> p k m", p=P)
                )

                a1s = apool.tile([P, HK, cap], f32r, tag="a1")
                yts = ypool.tile([P, CK, cap], f32, tag="yt")

                for n0, nsz in chunks:
                    # layer 1: A1T[m] = relu(sum_k W1[k,m]^T @ XgT[k] + b1[m])
                    for m in range(HK):
                        ps = pp.tile([P, 512], f32, tag="ps")
                        for k in range(FK):
                            nc.tensor.matmul(
                                ps[:, :nsz],
                                w1s[:, k, m * P:(m + 1) * P],
                                xts[:, k, n0:n0 + nsz],
                                start=(k == 0),
                                stop=(k == FK - 1),
                            )
                        nc.scalar.activation(
                            a1s[:, m, n0:n0 + nsz],
                            ps[:, :nsz],
                            mybir.ActivationFunctionType.Relu,
                            bias=b1s[:, e, m:m + 1],
                        )
                    # layer 2: YT[m] = sum_k W2[k,m]^T @ A1T[k] + b2[m]
                    for m in range(CK):
                        ps = pp.tile([P, 512], f32, tag="ps")
                        for k in range(HK):
                            nc.tensor.matmul(
                                ps[:, :nsz],
                                w2s[:, k, m * P:(m + 1) * P],
                                a1s[:, k, n0:n0 + nsz],
                                start=(k == 0),
                                stop=(k == HK - 1),
                            )
                        nc.vector.tensor_scalar_add(
                            yts[:, m, n0:n0 + nsz],
                            ps[:, :nsz],
                            b2s[:, e, m:m + 1],
                        )
                nc.sync.dma_start(
                    out=yt[e].rearrange("(k p) n -> p k n", p=P), in_=yts[:]
                )

    nc.compile()
    return nc


def _chunks(cap):
    n = (cap + 511) // 512
    base, rem = divmod(cap, n)
    out = []
    off = 0
    for i in range(n):
        sz = base + (1 if i < rem else 0)
        out.append((off, sz))
        off += sz
    return out


# ---------------------------------------------------------------------------
# entry point
# ---------------------------------------------------------------------------
def kernel(h, W_mu, b_mu, W_logvar, b_logvar, W1, b1, W2, b2):
    from concourse.bass_utils import run_bass_kernel_spmd

    h = np.ascontiguousarray(np.asarray(h, dtype=np.float32))
    W1 = np.asarray(W1, dtype=np.float32)
    b1 = np.asarray(b1, dtype=np.float32)
    W2 = np.asarray(W2, dtype=np.float32)
    b2 = np.asarray(b2, dtype=np.float32)

    B, F = h.shape
    E, _, H = W1.shape
    C = W2.shape[2]
    assert E % NCORES == 0
    epc = E // NCORES
    FK, HK, CK = F // P, H // P, C // P

    topk_idx, topk_w = _routing(
        np.asarray(h), np.asarray(W_mu), np.asarray(b_mu),
        np.asarray(W_logvar), np.asarray(b_logvar)
    )

    # per-expert token lists
    toks, poss = [], []
    for e in range(E):
        tok, pos = np.nonzero(topk_idx == e)
        toks.append(tok)
        poss.append(pos)
    maxcnt = max(len(t) for t in toks)
    cap = max(64, -(-maxcnt // 32) * 32)
    chunks = _chunks(cap)

    # gather/dispatch: XgT per expert, padded to cap
    xt = np.zeros((NCORES, epc, F, cap), np.float32)
    for e in range(E):
        cnt = len(toks[e])
        xt[e // epc, e % epc, :, :cnt] = h[toks[e]].T

    # biases pre-swizzled to [P, epc, K] so the device DMA is contiguous
    b1_sw = np.ascontiguousarray(
        b1.reshape(E, HK, P).transpose(2, 0, 1).reshape(P, NCORES, epc, HK)
        .transpose(1, 0, 2, 3)
    )  # [NCORES, P, epc, HK]
    b2_sw = np.ascontiguousarray(
        b2.reshape(E, CK, P).transpose(2, 0, 1).reshape(P, NCORES, epc, CK)
        .transpose(1, 0, 2, 3)
    )

    nc = _build_kernel(epc, F, H, C, cap, chunks)

    in_maps = []
    for c in range(NCORES):
        lo, hi = c * epc, (c + 1) * epc
        in_maps.append({
            "xt": xt[c],
            "w1": W1[lo:hi],
            "w2": W2[lo:hi],
            "b1": b1_sw[c],
            "b2": b2_sw[c],
        })

    trace = bool(os.environ.get("MOE_KERNEL_TRACE"))
    res = run_bass_kernel_spmd(nc, in_maps, list(range(NCORES)), trace=trace)
    global LAST_RESULTS
    LAST_RESULTS = res

    # combine: scatter-add weighted expert outputs
    out = np.zeros((B, C), np.float32)
    for e in range(E):
        cnt = len(toks[e])
        yte = res.results[e // epc]["yt"][e % epc]  # [C, cap]
        out[toks[e]] += (
            topk_w[toks[e], poss[e]].astype(np.float32)[:, None]
            * yte[:, :cnt].T
        )
    return out


LAST_RESULTS = None
